# revision 27
# baseline (speedup 1.0000x reference)
"""Trainium2 Bass kernel for nn_GCNGRU_Single (SAGEConv x2 on star graph -> 2-layer GRU -> FC).

Algebraic reduction (exact): only the hub node's sequence reaches the output, so
    seq[b,w,:] = (features[b,w,0,:] @ Wr1 + b1) @ Wr2 + b2      (Wl1/Wl2 unused)
    gi0        = seq @ Wih0.T + bih0 = hub_aug @ W_A            (bias via ones-row)

V3 device schedule (per core, batch 16, weights replicated). The per-beat serial
ring is the pacer; it is kept to 4 small ops plus a short matmul segment:

    mm(gi inject + Whh r,z) -> sigmoid -> scan1 -> tanh -> scan2 -> next beat

  * PSUM accumulation groups must be contiguous on the PE queue (measured:
    interleaving other groups between a stopped group and a start=False
    accumulate silently drops the old content), so gi_r/gi_z are injected with
    an I @ GI0 matmul immediately before each Whh matmul, baseline-style.
    The n gate needs NO accumulation: gin lands in its own stride-3 slots.
  * scan1 uses stride-3 groups  d0 = [0, 1, r], d1 = [ghn, 0, gin]:
        y2 = r*ghn + gin = an   (an at offset 2, same offset as r/z in masks)
  * the (h - n) subtract is folded into scan2 via stride-3 groups
        d0 = [0, -1, z]   d1 = [n, h, n]   ->  y2 = z*(h-n) + n = h'
    with tanh double-writing n (stride-0 broadcast read) and the h copy done
    on the Pool engine off the critical path.
  * one sigmoid writes both masks: out = M[:, :, 2::3]  (r row 0, z row 1).
  * L1 lags two beats; its ops fill engine slack behind L0's ring. Its input
    projection pairs (Wih1 first, start=True) execute before h1 arrives.

PSUM beat tile [H, 80] f32: [0:16) a_r | [16:32) a_z | [32:80) n-groups
(3b+32: ghn, 3b+33: 0 (memset once), 3b+34: gin).
"""

import sys

import numpy as np

for _p in ("/opt/trn_rl_repo", "/opt/pypackages"):
    if _p not in sys.path:
        sys.path.append(_p)

B, W, S, F, H, HOR = 128, 64, 64, 64, 128, 12
NCORES = 8
BL = B // NCORES  # 16 batch items per core
FA = F + 1        # hub features + ones row (bias folding)

# Recover the axon terminal if a previous process left a wedged NRT exec unit.
try:
    import ctypes as _ct

    _ct.CDLL("/opt/axon/libaxon_pjrt.so").axon_reset()
except Exception:
    pass

_BUILD_CACHE: dict = {}


def _build_nc(flags):
    import concourse.bacc as bacc
    import concourse.tile as tile
    from concourse import mybir
    from concourse.tile import add_dep_helper

    bhh0n_nz, b1rz_nz, bih1n_nz, bhh1n_nz = flags
    f32 = mybir.dt.float32
    f16 = mybir.dt.float16
    Sig = mybir.ActivationFunctionType.Sigmoid
    Tanh = mybir.ActivationFunctionType.Tanh
    Ident = mybir.ActivationFunctionType.Identity
    MUL = mybir.AluOpType.mult
    ADD = mybir.AluOpType.add

    nc = bacc.Bacc("TRN2", target_bir_lowering=False, debug=False,
                   enable_asserts=False, num_devices=NCORES)

    need_brep = bhh0n_nz or b1rz_nz or bih1n_nz or bhh1n_nz
    hubA_d = nc.dram_tensor("hubA", [FA, W * BL], f16, kind="ExternalInput")
    WA_d = nc.dram_tensor("WA", [FA, 3 * H], f16, kind="ExternalInput")
    WPK = 3 * (3 * H) + H + HOR  # Whh0T | Wih1T | Whh1T | I128 | Wfc
    wpack_d = nc.dram_tensor("wpack", [H, WPK], f16, kind="ExternalInput")
    bfc_d = nc.dram_tensor("bfc", [HOR, 1], f32, kind="ExternalInput")
    if need_brep:
        brep_d = nc.dram_tensor("brep", [H, 5 * BL], f16, kind="ExternalInput")
    out_d = nc.dram_tensor("out", [HOR, BL], f32, kind="ExternalOutput")

    CH1 = 8 * BL                          # hubA columns DMA'd first

    with tile.TileContext(nc) as tc:
        with (
            tc.tile_pool(name="weights", bufs=1) as wpool,
            tc.tile_pool(name="state", bufs=4) as hpool,
            tc.tile_pool(name="an", bufs=3) as anpool,
            tc.tile_pool(name="un", bufs=3) as unpool,
            tc.tile_pool(name="psBeat", bufs=1, space="PSUM") as psBeat,
            tc.tile_pool(name="psPre", bufs=3, space="PSUM") as psPre,
        ):
            hubA = wpool.tile([FA, W * BL], f16, tag="hubA")
            WA = wpool.tile([FA, 3 * H], f16, tag="WA")
            wpack = wpool.tile([H, WPK], f16, tag="wpack")
            Whh0T = wpack[:, 0:3 * H]
            Wih1T = wpack[:, 3 * H:6 * H]
            Whh1T = wpack[:, 6 * H:9 * H]
            I128 = wpack[:, 9 * H:10 * H]
            Wfc = wpack[:, 10 * H:10 * H + HOR]
            bfc = wpool.tile([HOR, 1], f32, tag="bfc")
            GI0r = wpool.tile([H, W * BL], f16, tag="GI0r")
            GI0z = wpool.tile([H, W * BL], f16, tag="GI0z")
            GI0n = wpool.tile([H, W * BL], f16, tag="GI0n")
            if need_brep:
                brep = wpool.tile([H, 5 * BL], f16, tag="brep")

            # one packed weight DMA per queue; transfers gate beat 0
            # order transfers by when the program first needs them
            nc.sync.dma_start(out=hubA[:, 0:CH1], in_=hubA_d[:, 0:CH1])
            nc.sync.dma_start(out=WA[:], in_=WA_d[:])
            nc.sync.dma_start(out=hubA[:, CH1:], in_=hubA_d[:, CH1:])
            nc.gpsimd.dma_start(out=wpack[:, 9 * H:], in_=wpack_d[:, 9 * H:])
            nc.gpsimd.dma_start(out=wpack[:, 0:3 * H], in_=wpack_d[:, 0:3 * H])
            nc.gpsimd.dma_start(out=wpack[:, 3 * H:9 * H], in_=wpack_d[:, 3 * H:9 * H])
            nc.gpsimd.dma_start(out=bfc[:], in_=bfc_d[:])
            if need_brep:
                nc.gpsimd.dma_start(out=brep[:], in_=brep_d[:])

            # masks: row0 = [0, 1, r], row1 = [0, -1, z] per group of 3
            M0 = [wpool.tile([H, 2, 3 * BL], f16, tag=f"M0{i}", name=f"M0{i}")
                  for i in range(2)]
            M1 = [wpool.tile([H, 2, 3 * BL], f16, tag=f"M1{i}", name=f"M1{i}")
                  for i in range(2)]
            for m in M0 + M1:
                nc.vector.memset(m[:], 0.0)
                nc.vector.memset(m[:, 0, 1:3 * BL:3], 1.0)
                nc.vector.memset(m[:, 1, 1:3 * BL:3], -1.0)

            h_init = hpool.tile([H, 3 * BL], f16, tag="hinit", bufs=1)
            nc.vector.memset(h_init[:], 0.0)

            # per-beat PSUM tiles, manually rotated; 33+3b slots stay 0 forever
            P0s = [psBeat.tile([H, 80], f32, tag=f"P0_{i}", name=f"P0_{i}")
                   for i in range(2)]
            P1s = [psBeat.tile([H, 80], f32, tag=f"P1_{i}", name=f"P1_{i}")
                   for i in range(2)]
            for p in P0s + P1s:
                nc.vector.memset(p[:, 33:80:3], 0.0)

            # ---- GI0 precompute: PSUM -> SBUF fp16 (bias via ones-row) ----
            # chunk 0 (steps 0-7) runs before the loop; the remaining 8-step
            # chunks are interleaved into early beats' engine slack below.
            ring_anchor: dict = {}

            def pre_mm(c, g, anchor=None):
                w0c, w1c = 8 * c, 8 * (c + 1)
                ncols = (w1c - w0c) * BL
                pg = psPre.tile([H, 128], f32, tag="pre", name="pg")
                i_mm = nc.tensor.matmul(
                    out=pg[:, 0:ncols],
                    lhsT=WA[:, g * H:(g + 1) * H],
                    rhs=hubA[:, w0c * BL:w1c * BL],
                    start=True, stop=True,
                )
                if anchor is not None:
                    add_dep_helper(i_mm.ins, anchor.ins, sync=True,
                                   reason="precompute chunks stay behind the ring")
                return pg

            def pre_copy(c, g, pg):
                dst = (GI0r, GI0z, GI0n)[g]
                nc.vector.tensor_scalar_add(
                    dst[:, 8 * c * BL:8 * (c + 1) * BL], pg[:, 0:8 * BL], 0.0)

            for g in range(3):
                pg = pre_mm(0, g)
                pre_copy(0, g, pg)

            pre_sched: dict = {}  # beat -> list of emit thunks
            for c in range(1, 8):
                b0 = 1 + (c - 1) * 3
                pgs = {}
                def mk_mm(c=c, b0=b0):
                    def f():
                        anchor = ring_anchor.get(b0 - 1)
                        for g in range(3):
                            pgs[g] = pre_mm(c, g, anchor if g == 0 else None)
                    return f
                pre_sched.setdefault(b0, []).append(mk_mm())
                for g in range(3):
                    def mk_cp(c=c, g=g):
                        return lambda: pre_copy(c, g, pgs[g])
                    pre_sched.setdefault(b0 + g, []).append(mk_cp())

            h0_prev = h_init
            h0_pprev = h_init
            h1_prev = h_init

            for u in range(W + 2):
                do0 = u < W
                do1 = u >= 2
                if do0:
                    P0 = P0s[u % 2]
                    h0r = h0_prev[:, 2:3 * BL:3]
                    c0 = slice(u * BL, (u + 1) * BL)
                    # r, z: contiguous (inject, Whh) accumulation pairs
                    for g, (blk, gi) in enumerate(((slice(0, BL), GI0r),
                                                   (slice(BL, 2 * BL), GI0z))):
                        nc.tensor.matmul(out=P0[:, blk], lhsT=I128,
                                         rhs=gi[:, c0], start=True, stop=False,
                                         skip_group_check=True)
                        nc.tensor.matmul(out=P0[:, blk],
                                         lhsT=Whh0T[:, g * H:(g + 1) * H],
                                         rhs=h0r, start=False, stop=True,
                                         skip_group_check=True)
                    # n: ghn and gin have their own slots, no accumulation
                    if bhh0n_nz:
                        nc.tensor.matmul(out=P0[:, 32:80:3], lhsT=I128,
                                         rhs=brep[:, 0:BL], start=True, stop=False,
                                         skip_group_check=True)
                    nc.tensor.matmul(out=P0[:, 32:80:3], lhsT=Whh0T[:, 2 * H:3 * H],
                                     rhs=h0r, start=not bhh0n_nz, stop=True,
                                     skip_group_check=True)
                    nc.tensor.matmul(out=P0[:, 34:80:3], lhsT=I128,
                                     rhs=GI0n[:, c0], start=True, stop=True,
                                     skip_group_check=True)
                if do1:
                    P1 = P1s[u % 2]
                    h1r = h1_prev[:, 2:3 * BL:3]
                    h0x = h0_pprev[:, 2:3 * BL:3]
                    for g in range(2):
                        blk = slice(g * BL, (g + 1) * BL)
                        # Wih1 part first: its operand is 2 beats old
                        nc.tensor.matmul(out=P1[:, blk],
                                         lhsT=Wih1T[:, g * H:(g + 1) * H],
                                         rhs=h0x, start=True, stop=False,
                                         skip_group_check=True)
                        if b1rz_nz:
                            nc.tensor.matmul(out=P1[:, blk], lhsT=I128,
                                             rhs=brep[:, (1 + g) * BL:(2 + g) * BL],
                                             start=False, stop=False,
                                             skip_group_check=True)
                        nc.tensor.matmul(out=P1[:, blk],
                                         lhsT=Whh1T[:, g * H:(g + 1) * H],
                                         rhs=h1r, start=False, stop=True,
                                         skip_group_check=True)
                    if bhh1n_nz:
                        nc.tensor.matmul(out=P1[:, 32:80:3], lhsT=I128,
                                         rhs=brep[:, 4 * BL:5 * BL], start=True,
                                         stop=False, skip_group_check=True)
                    nc.tensor.matmul(out=P1[:, 32:80:3], lhsT=Whh1T[:, 2 * H:3 * H],
                                     rhs=h1r, start=not bhh1n_nz, stop=True,
                                     skip_group_check=True)
                    nc.tensor.matmul(out=P1[:, 34:80:3], lhsT=Wih1T[:, 2 * H:3 * H],
                                     rhs=h0x, start=True, stop=not bih1n_nz,
                                     skip_group_check=True)
                    if bih1n_nz:
                        nc.tensor.matmul(out=P1[:, 34:80:3], lhsT=I128,
                                         rhs=brep[:, 3 * BL:4 * BL], start=False,
                                         stop=True, skip_group_check=True)

                for thunk in pre_sched.pop(u, ()):
                    thunk()

                # h copies into the scan2 data tiles (Pool, off the ring)
                if do0:
                    un0 = unpool.tile([H, 3 * BL], f16, tag="un0", name="un0")
                    nc.gpsimd.tensor_scalar_add(un0[:, 1:3 * BL:3],
                                                h0_prev[:, 2:3 * BL:3], 0.0)
                if do1:
                    un1 = unpool.tile([H, 3 * BL], f16, tag="un1", name="un1")
                    nc.gpsimd.tensor_scalar_add(un1[:, 1:3 * BL:3],
                                                h1_prev[:, 2:3 * BL:3], 0.0)

                def gate_math(P, m, un, an_tag, h_tag, sc2_eng=nc.vector):
                    i_sig = nc.scalar.activation(
                        out=m[:, :, 2:3 * BL:3],
                        in_=P[:, 0:2 * BL].rearrange("p (g b) -> p g b", g=2),
                        func=Sig)
                    an = anpool.tile([H, 3 * BL], f32, tag=an_tag, name=an_tag)
                    nc.vector.tensor_tensor_scan(
                        out=an[:], data0=m[:, 0, :], data1=P[:, 32:80],
                        initial=0.0, op0=MUL, op1=ADD)
                    i_tanh = nc.scalar.activation(
                        out=un.rearrange("p (b t) -> p b t", t=3)[:, :, 0:3:2],
                        in_=an[:, 2:3 * BL:3].unsqueeze(2).broadcast_to([H, BL, 2]),
                        func=Tanh)
                    h_new = hpool.tile([H, 3 * BL], f16, tag=h_tag, name=h_tag)
                    i_sc2 = sc2_eng.tensor_tensor_scan(
                        out=h_new[:], data0=m[:, 1, :], data1=un[:],
                        initial=0.0, op0=MUL, op1=ADD)
                    return h_new, (i_sig, i_tanh), i_sc2

                acts0 = acts1 = None
                if do0:
                    h0_new, acts0, i_sc2 = gate_math(P0, M0[u % 2], un0, "an0", "h0")
                    ring_anchor[u] = i_sc2
                if do1:
                    h1_new, acts1, _ = gate_math(P1, M1[u % 2], un1, "an1", "h1")
                if acts0 is not None and acts1 is not None:
                    add_dep_helper(acts1[0].ins, acts0[1].ins, sync=True,
                                   reason="L0 chain priority on Scalar")

                if do0:
                    h0_pprev = h0_prev
                    h0_prev = h0_new
                else:
                    h0_pprev = h0_prev
                if do1:
                    h1_prev = h1_new

            # ---- final FC: out = Wfc.T @ h1 + bfc ----
            with tc.tile_pool(name="psFC", bufs=1, space="PSUM") as psFC:
                pfc = psFC.tile([HOR, BL], f32, tag="fc")
                nc.tensor.matmul(out=pfc[:], lhsT=Wfc,
                                 rhs=h1_prev[:, 2:3 * BL:3], start=True, stop=True)
                t_out = anpool.tile([HOR, BL], f32, tag="out")
                nc.scalar.activation(out=t_out[:], in_=pfc[:], func=Ident,
                                     bias=bfc[:, 0:1])
                nc.sync.dma_start(out=out_d[:], in_=t_out[:])

    nc.compile()
    return nc


def _host_prep(inputs):
    """Fold weights on host (float64 folds), build per-core input maps."""
    fx = np.asarray(inputs["features"], np.float32)
    Wr1 = np.asarray(inputs["Wr1"], np.float64)
    Wr2 = np.asarray(inputs["Wr2"], np.float64)
    b1 = np.asarray(inputs["b1"], np.float64)
    b2 = np.asarray(inputs["b2"], np.float64)
    Wih0 = np.asarray(inputs["Wih0"], np.float64)
    bih0 = np.asarray(inputs["bih0"], np.float64)
    bhh0 = np.asarray(inputs["bhh0"], np.float64)
    Wih1 = np.asarray(inputs["Wih1"], np.float32)
    Whh0 = np.asarray(inputs["Whh0"], np.float32)
    Whh1 = np.asarray(inputs["Whh1"], np.float32)
    bih1 = np.asarray(inputs["bih1"], np.float64)
    bhh1 = np.asarray(inputs["bhh1"], np.float64)
    Wfc = np.asarray(inputs["Wfc"], np.float32)
    bfc = np.asarray(inputs["bfc"], np.float32)

    W12 = Wr1 @ Wr2                       # [F, H]
    bias12 = b1 @ Wr2 + b2                # [H]
    W_A = W12 @ Wih0.T                    # [F, 3H]
    b_A = (bias12 @ Wih0.T + bih0).copy()  # [3H]
    b_A[0:H] += bhh0[0:H]
    b_A[H:2 * H] += bhh0[H:2 * H]
    WA_aug = np.vstack([W_A, b_A[None, :]]).astype(np.float16)  # [FA, 3H]

    brep = np.zeros((H, 5 * BL), np.float16)
    brep[:, 0 * BL:1 * BL] = bhh0[2 * H:3 * H, None]
    brep[:, 1 * BL:2 * BL] = (bih1[0:H] + bhh1[0:H])[:, None]
    brep[:, 2 * BL:3 * BL] = (bih1[H:2 * H] + bhh1[H:2 * H])[:, None]
    brep[:, 3 * BL:4 * BL] = bih1[2 * H:3 * H, None]
    brep[:, 4 * BL:5 * BL] = bhh1[2 * H:3 * H, None]

    flags = (
        bool(np.any(brep[:, 0:BL] != 0)),
        bool(np.any(brep[:, BL:3 * BL] != 0)),
        bool(np.any(brep[:, 3 * BL:4 * BL] != 0)),
        bool(np.any(brep[:, 4 * BL:5 * BL] != 0)),
    )
    need_brep = any(flags)

    wpack = np.concatenate([
        Whh0.T.astype(np.float16),
        Wih1.T.astype(np.float16),
        Whh1.T.astype(np.float16),
        np.eye(H, dtype=np.float16),
        Wfc.astype(np.float16),
    ], axis=1)
    shared = {
        "WA": np.ascontiguousarray(WA_aug),
        "wpack": np.ascontiguousarray(wpack),
        "bfc": np.ascontiguousarray(bfc.reshape(HOR, 1)),
    }
    if need_brep:
        shared["brep"] = brep

    hub = fx[:, :, 0, :]                  # [B, W, F]
    in_maps = []
    for c in range(NCORES):
        hub_c = hub[c * BL:(c + 1) * BL]  # [BL, W, F]
        hubT = hub_c.transpose(2, 1, 0).reshape(F, W * BL).astype(np.float16)
        hubA = np.vstack([hubT, np.ones((1, W * BL), np.float16)])
        in_maps.append({"hubA": np.ascontiguousarray(hubA), **shared})
    return in_maps, flags


def kernel(**inputs) -> np.ndarray:
    from concourse.bass_utils import run_bass_kernel_spmd

    in_maps, flags = _host_prep(inputs)
    if flags not in _BUILD_CACHE:
        _BUILD_CACHE[flags] = _build_nc(flags)
    nc = _BUILD_CACHE[flags]

    res = run_bass_kernel_spmd(nc, in_maps, core_ids=list(range(NCORES)))
    out = np.empty((B, HOR), np.float32)
    for c in range(NCORES):
        out[c * BL:(c + 1) * BL] = res.results[c]["out"].T
    return out


# revision 28
# speedup vs baseline: 1.0232x; 1.0232x over previous
"""Trainium2 Bass kernel for nn_GCNGRU_Single (SAGEConv x2 on star graph -> 2-layer GRU -> FC).

Algebraic reduction (exact): only the hub node's sequence reaches the output, so
    seq[b,w,:] = (features[b,w,0,:] @ Wr1 + b1) @ Wr2 + b2      (Wl1/Wl2 unused)
    gi0        = seq @ Wih0.T + bih0 = hub_aug @ W_A            (bias via ones-row)

V3 device schedule (per core, batch 16, weights replicated). The per-beat serial
ring is the pacer; it is kept to 4 small ops plus a short matmul segment:

    mm(gi inject + Whh r,z) -> sigmoid -> scan1 -> tanh -> scan2 -> next beat

  * PSUM accumulation groups must be contiguous on the PE queue (measured:
    interleaving other groups between a stopped group and a start=False
    accumulate silently drops the old content), so gi_r/gi_z are injected with
    an I @ GI0 matmul immediately before each Whh matmul, baseline-style.
    The n gate needs NO accumulation: gin lands in its own stride-3 slots.
  * scan1 uses stride-3 groups  d0 = [0, 1, r], d1 = [ghn, 0, gin]:
        y2 = r*ghn + gin = an   (an at offset 2, same offset as r/z in masks)
  * the (h - n) subtract is folded into scan2 via stride-3 groups
        d0 = [0, -1, z]   d1 = [n, h, n]   ->  y2 = z*(h-n) + n = h'
    with tanh double-writing n (stride-0 broadcast read) and the h copy done
    on the Pool engine off the critical path.
  * one sigmoid writes both masks: out = M[:, :, 2::3]  (r row 0, z row 1).
  * L1 lags two beats; its ops fill engine slack behind L0's ring. Its input
    projection pairs (Wih1 first, start=True) execute before h1 arrives.

PSUM beat tile [H, 80] f32: [0:16) a_r | [16:32) a_z | [32:80) n-groups
(3b+32: ghn, 3b+33: 0 (memset once), 3b+34: gin).
"""

import sys

import numpy as np

for _p in ("/opt/trn_rl_repo", "/opt/pypackages"):
    if _p not in sys.path:
        sys.path.append(_p)

B, W, S, F, H, HOR = 128, 64, 64, 64, 128, 12
NCORES = 8
BL = B // NCORES  # 16 batch items per core
FA = F + 1        # hub features + ones row (bias folding)

# Recover the axon terminal if a previous process left a wedged NRT exec unit.
try:
    import ctypes as _ct

    _ct.CDLL("/opt/axon/libaxon_pjrt.so").axon_reset()
except Exception:
    pass

_BUILD_CACHE: dict = {}


def _build_nc(flags):
    import concourse.bacc as bacc
    import concourse.tile as tile
    from concourse import mybir
    from concourse.tile import add_dep_helper

    bhh0n_nz, b1rz_nz, bih1n_nz, bhh1n_nz = flags
    f32 = mybir.dt.float32
    f16 = mybir.dt.float16
    Sig = mybir.ActivationFunctionType.Sigmoid
    Tanh = mybir.ActivationFunctionType.Tanh
    Ident = mybir.ActivationFunctionType.Identity
    MUL = mybir.AluOpType.mult
    ADD = mybir.AluOpType.add

    nc = bacc.Bacc("TRN2", target_bir_lowering=False, debug=False,
                   enable_asserts=False, num_devices=NCORES)

    need_brep = bhh0n_nz or b1rz_nz or bih1n_nz or bhh1n_nz
    hubA_d = nc.dram_tensor("hubA", [FA, W * BL], f16, kind="ExternalInput")
    WA_d = nc.dram_tensor("WA", [FA, 3 * H], f16, kind="ExternalInput")
    WPK = 3 * (3 * H) + H + HOR  # Whh0T | Wih1T | Whh1T | I128 | Wfc
    wpack_d = nc.dram_tensor("wpack", [H, WPK], f16, kind="ExternalInput")
    bfc_d = nc.dram_tensor("bfc", [HOR, 1], f32, kind="ExternalInput")
    if need_brep:
        brep_d = nc.dram_tensor("brep", [H, 5 * BL], f16, kind="ExternalInput")
    out_d = nc.dram_tensor("out", [HOR, BL], f32, kind="ExternalOutput")

    CH1 = 8 * BL                          # hubA columns DMA'd first

    with tile.TileContext(nc) as tc:
        with (
            tc.tile_pool(name="weights", bufs=1) as wpool,
            tc.tile_pool(name="state", bufs=4) as hpool,
            tc.tile_pool(name="an", bufs=3) as anpool,
            tc.tile_pool(name="un", bufs=3) as unpool,
            tc.tile_pool(name="psBeat", bufs=1, space="PSUM") as psBeat,
            tc.tile_pool(name="psPre", bufs=3, space="PSUM") as psPre,
        ):
            hubA = wpool.tile([FA, W * BL], f16, tag="hubA")
            WA = wpool.tile([FA, 3 * H], f16, tag="WA")
            wpack = wpool.tile([H, WPK], f16, tag="wpack")
            Whh0T = wpack[:, 0:3 * H]
            Wih1T = wpack[:, 3 * H:6 * H]
            Whh1T = wpack[:, 6 * H:9 * H]
            I128 = wpack[:, 9 * H:10 * H]
            Wfc = wpack[:, 10 * H:10 * H + HOR]
            bfc = wpool.tile([HOR, 1], f32, tag="bfc")
            GI0r = wpool.tile([H, W * BL], f16, tag="GI0r")
            GI0z = wpool.tile([H, W * BL], f16, tag="GI0z")
            GI0n = wpool.tile([H, W * BL], f16, tag="GI0n")
            if need_brep:
                brep = wpool.tile([H, 5 * BL], f16, tag="brep")

            # one packed weight DMA per queue; transfers gate beat 0
            # order transfers by when the program first needs them
            nc.sync.dma_start(out=hubA[:, 0:CH1], in_=hubA_d[:, 0:CH1])
            nc.sync.dma_start(out=WA[:], in_=WA_d[:])
            nc.sync.dma_start(out=hubA[:, CH1:], in_=hubA_d[:, CH1:])
            nc.gpsimd.dma_start(out=wpack[:, 9 * H:], in_=wpack_d[:, 9 * H:])
            nc.gpsimd.dma_start(out=wpack[:, 0:3 * H], in_=wpack_d[:, 0:3 * H])
            nc.gpsimd.dma_start(out=wpack[:, 3 * H:9 * H], in_=wpack_d[:, 3 * H:9 * H])
            nc.gpsimd.dma_start(out=bfc[:], in_=bfc_d[:])
            if need_brep:
                nc.gpsimd.dma_start(out=brep[:], in_=brep_d[:])

            # masks: row0 = [0, 1, r], row1 = [0, -1, z] per group of 3
            M0 = [wpool.tile([H, 2, 3 * BL], f16, tag=f"M0{i}", name=f"M0{i}")
                  for i in range(2)]
            M1 = [wpool.tile([H, 2, 3 * BL], f16, tag=f"M1{i}", name=f"M1{i}")
                  for i in range(2)]
            for m in M0 + M1:
                nc.vector.memset(m[:], 0.0)
                nc.vector.memset(m[:, 0, 1:3 * BL:3], 1.0)
                nc.vector.memset(m[:, 1, 1:3 * BL:3], -1.0)

            h_init = hpool.tile([H, 3 * BL], f16, tag="hinit", bufs=1)
            nc.vector.memset(h_init[:], 0.0)

            # per-beat PSUM tiles, manually rotated; 33+3b slots stay 0 forever
            P0s = [psBeat.tile([H, 80], f32, tag=f"P0_{i}", name=f"P0_{i}")
                   for i in range(2)]
            P1s = [psBeat.tile([H, 80], f32, tag=f"P1_{i}", name=f"P1_{i}")
                   for i in range(2)]
            for p in P0s + P1s:
                nc.vector.memset(p[:, 33:80:3], 0.0)

            # ---- GI0 precompute: PSUM -> SBUF fp16 (bias via ones-row) ----
            # chunk 0 (steps 0-7) runs before the loop; the remaining 8-step
            # chunks are interleaved into early beats' engine slack below.
            ring_anchor: dict = {}

            def pre_mm(c, g, anchor=None):
                w0c, w1c = 8 * c, 8 * (c + 1)
                ncols = (w1c - w0c) * BL
                pg = psPre.tile([H, 128], f32, tag="pre", name="pg")
                i_mm = nc.tensor.matmul(
                    out=pg[:, 0:ncols],
                    lhsT=WA[:, g * H:(g + 1) * H],
                    rhs=hubA[:, w0c * BL:w1c * BL],
                    start=True, stop=True,
                )
                if anchor is not None:
                    add_dep_helper(i_mm.ins, anchor.ins, sync=True,
                                   reason="precompute chunks stay behind the ring")
                return pg

            def pre_copy(c, g, pg):
                dst = (GI0r, GI0z, GI0n)[g]
                nc.vector.tensor_scalar_add(
                    dst[:, 8 * c * BL:8 * (c + 1) * BL], pg[:, 0:8 * BL], 0.0)

            for g in range(3):
                pg = pre_mm(0, g)
                pre_copy(0, g, pg)

            pre_sched: dict = {}  # beat -> list of emit thunks
            for c in range(1, 8):
                b0 = 1 + (c - 1) * 3
                pgs = {}
                def mk_mm(c=c, b0=b0):
                    def f():
                        anchor = ring_anchor.get(b0 - 1)
                        for g in range(3):
                            pgs[g] = pre_mm(c, g, anchor if g == 0 else None)
                    return f
                pre_sched.setdefault(b0, []).append(mk_mm())
                for g in range(3):
                    def mk_cp(c=c, g=g):
                        return lambda: pre_copy(c, g, pgs[g])
                    pre_sched.setdefault(b0 + g, []).append(mk_cp())

            h0_prev = h_init
            h0_pprev = h_init
            h1_prev = h_init

            for u in range(W + 2):
                do0 = u < W
                do1 = u >= 2
                if do0:
                    P0 = P0s[u % 2]
                    h0r = h0_prev[:, 2:3 * BL:3]
                    c0 = slice(u * BL, (u + 1) * BL)
                    # r, z: contiguous (inject, Whh) accumulation pairs
                    for g, (blk, gi) in enumerate(((slice(0, BL), GI0r),
                                                   (slice(BL, 2 * BL), GI0z))):
                        nc.tensor.matmul(out=P0[:, blk], lhsT=I128,
                                         rhs=gi[:, c0], start=True, stop=False,
                                         skip_group_check=True)
                        nc.tensor.matmul(out=P0[:, blk],
                                         lhsT=Whh0T[:, g * H:(g + 1) * H],
                                         rhs=h0r, start=False, stop=True,
                                         skip_group_check=True)
                    # n: ghn and gin have their own slots, no accumulation
                    if bhh0n_nz:
                        nc.tensor.matmul(out=P0[:, 32:80:3], lhsT=I128,
                                         rhs=brep[:, 0:BL], start=True, stop=False,
                                         skip_group_check=True)
                    nc.tensor.matmul(out=P0[:, 32:80:3], lhsT=Whh0T[:, 2 * H:3 * H],
                                     rhs=h0r, start=not bhh0n_nz, stop=True,
                                     skip_group_check=True)
                    nc.tensor.matmul(out=P0[:, 34:80:3], lhsT=I128,
                                     rhs=GI0n[:, c0], start=True, stop=True,
                                     skip_group_check=True)
                if do1:
                    P1 = P1s[u % 2]
                    h1r = h1_prev[:, 2:3 * BL:3]
                    h0x = h0_pprev[:, 2:3 * BL:3]
                    for g in range(2):
                        blk = slice(g * BL, (g + 1) * BL)
                        # Wih1 part first: its operand is 2 beats old
                        nc.tensor.matmul(out=P1[:, blk],
                                         lhsT=Wih1T[:, g * H:(g + 1) * H],
                                         rhs=h0x, start=True, stop=False,
                                         skip_group_check=True)
                        if b1rz_nz:
                            nc.tensor.matmul(out=P1[:, blk], lhsT=I128,
                                             rhs=brep[:, (1 + g) * BL:(2 + g) * BL],
                                             start=False, stop=False,
                                             skip_group_check=True)
                        nc.tensor.matmul(out=P1[:, blk],
                                         lhsT=Whh1T[:, g * H:(g + 1) * H],
                                         rhs=h1r, start=False, stop=True,
                                         skip_group_check=True)
                    if bhh1n_nz:
                        nc.tensor.matmul(out=P1[:, 32:80:3], lhsT=I128,
                                         rhs=brep[:, 4 * BL:5 * BL], start=True,
                                         stop=False, skip_group_check=True)
                    nc.tensor.matmul(out=P1[:, 32:80:3], lhsT=Whh1T[:, 2 * H:3 * H],
                                     rhs=h1r, start=not bhh1n_nz, stop=True,
                                     skip_group_check=True)
                    nc.tensor.matmul(out=P1[:, 34:80:3], lhsT=Wih1T[:, 2 * H:3 * H],
                                     rhs=h0x, start=True, stop=not bih1n_nz,
                                     skip_group_check=True)
                    if bih1n_nz:
                        nc.tensor.matmul(out=P1[:, 34:80:3], lhsT=I128,
                                         rhs=brep[:, 3 * BL:4 * BL], start=False,
                                         stop=True, skip_group_check=True)

                for thunk in pre_sched.pop(u, ()):
                    thunk()

                # h copies into the scan2 data tiles (Pool, off the ring)
                if do0:
                    un0 = unpool.tile([H, 3 * BL], f16, tag="un0", name="un0")
                    nc.gpsimd.tensor_scalar_add(un0[:, 1:3 * BL:3],
                                                h0_prev[:, 2:3 * BL:3], 0.0)
                if do1:
                    un1 = unpool.tile([H, 3 * BL], f16, tag="un1", name="un1")
                    nc.gpsimd.tensor_scalar_add(un1[:, 1:3 * BL:3],
                                                h1_prev[:, 2:3 * BL:3], 0.0)

                def gate_math(P, m, un, an_tag, h_tag, sc2_eng=nc.vector):
                    i_sig = nc.scalar.activation(
                        out=m[:, :, 2:3 * BL:3],
                        in_=P[:, 0:2 * BL].rearrange("p (g b) -> p g b", g=2),
                        func=Sig)
                    an = anpool.tile([H, 3 * BL], f32, tag=an_tag, name=an_tag)
                    nc.vector.tensor_tensor_scan(
                        out=an[:], data0=m[:, 0, :], data1=P[:, 32:80],
                        initial=0.0, op0=MUL, op1=ADD)
                    i_tanh = nc.scalar.activation(
                        out=un.rearrange("p (b t) -> p b t", t=3)[:, :, 0:3:2],
                        in_=an[:, 2:3 * BL:3].unsqueeze(2).broadcast_to([H, BL, 2]),
                        func=Tanh)
                    h_new = hpool.tile([H, 3 * BL], f16, tag=h_tag, name=h_tag)
                    i_sc2 = sc2_eng.tensor_tensor_scan(
                        out=h_new[:], data0=m[:, 1, :], data1=un[:],
                        initial=0.0, op0=MUL, op1=ADD)
                    return h_new, (i_sig, i_tanh), i_sc2

                # NOTE: no explicit Scalar-ordering dep here — the scheduler
                # already orders sig0, tanh1(u-1), tanh0, sig1 well, and the
                # extra semaphore cost ~55ns/beat (measured).
                if do0:
                    h0_new, _, i_sc2 = gate_math(P0, M0[u % 2], un0, "an0", "h0")
                    ring_anchor[u] = i_sc2
                if do1:
                    h1_new, _, _ = gate_math(P1, M1[u % 2], un1, "an1", "h1")

                if do0:
                    h0_pprev = h0_prev
                    h0_prev = h0_new
                else:
                    h0_pprev = h0_prev
                if do1:
                    h1_prev = h1_new

            # ---- final FC: out = Wfc.T @ h1 + bfc ----
            with tc.tile_pool(name="psFC", bufs=1, space="PSUM") as psFC:
                pfc = psFC.tile([HOR, BL], f32, tag="fc")
                nc.tensor.matmul(out=pfc[:], lhsT=Wfc,
                                 rhs=h1_prev[:, 2:3 * BL:3], start=True, stop=True)
                t_out = anpool.tile([HOR, BL], f32, tag="out")
                nc.scalar.activation(out=t_out[:], in_=pfc[:], func=Ident,
                                     bias=bfc[:, 0:1])
                nc.sync.dma_start(out=out_d[:], in_=t_out[:])

    nc.compile()
    return nc


def _host_prep(inputs):
    """Fold weights on host (float64 folds), build per-core input maps."""
    fx = np.asarray(inputs["features"], np.float32)
    Wr1 = np.asarray(inputs["Wr1"], np.float64)
    Wr2 = np.asarray(inputs["Wr2"], np.float64)
    b1 = np.asarray(inputs["b1"], np.float64)
    b2 = np.asarray(inputs["b2"], np.float64)
    Wih0 = np.asarray(inputs["Wih0"], np.float64)
    bih0 = np.asarray(inputs["bih0"], np.float64)
    bhh0 = np.asarray(inputs["bhh0"], np.float64)
    Wih1 = np.asarray(inputs["Wih1"], np.float32)
    Whh0 = np.asarray(inputs["Whh0"], np.float32)
    Whh1 = np.asarray(inputs["Whh1"], np.float32)
    bih1 = np.asarray(inputs["bih1"], np.float64)
    bhh1 = np.asarray(inputs["bhh1"], np.float64)
    Wfc = np.asarray(inputs["Wfc"], np.float32)
    bfc = np.asarray(inputs["bfc"], np.float32)

    W12 = Wr1 @ Wr2                       # [F, H]
    bias12 = b1 @ Wr2 + b2                # [H]
    W_A = W12 @ Wih0.T                    # [F, 3H]
    b_A = (bias12 @ Wih0.T + bih0).copy()  # [3H]
    b_A[0:H] += bhh0[0:H]
    b_A[H:2 * H] += bhh0[H:2 * H]
    WA_aug = np.vstack([W_A, b_A[None, :]]).astype(np.float16)  # [FA, 3H]

    brep = np.zeros((H, 5 * BL), np.float16)
    brep[:, 0 * BL:1 * BL] = bhh0[2 * H:3 * H, None]
    brep[:, 1 * BL:2 * BL] = (bih1[0:H] + bhh1[0:H])[:, None]
    brep[:, 2 * BL:3 * BL] = (bih1[H:2 * H] + bhh1[H:2 * H])[:, None]
    brep[:, 3 * BL:4 * BL] = bih1[2 * H:3 * H, None]
    brep[:, 4 * BL:5 * BL] = bhh1[2 * H:3 * H, None]

    flags = (
        bool(np.any(brep[:, 0:BL] != 0)),
        bool(np.any(brep[:, BL:3 * BL] != 0)),
        bool(np.any(brep[:, 3 * BL:4 * BL] != 0)),
        bool(np.any(brep[:, 4 * BL:5 * BL] != 0)),
    )
    need_brep = any(flags)

    wpack = np.concatenate([
        Whh0.T.astype(np.float16),
        Wih1.T.astype(np.float16),
        Whh1.T.astype(np.float16),
        np.eye(H, dtype=np.float16),
        Wfc.astype(np.float16),
    ], axis=1)
    shared = {
        "WA": np.ascontiguousarray(WA_aug),
        "wpack": np.ascontiguousarray(wpack),
        "bfc": np.ascontiguousarray(bfc.reshape(HOR, 1)),
    }
    if need_brep:
        shared["brep"] = brep

    hub = fx[:, :, 0, :]                  # [B, W, F]
    in_maps = []
    for c in range(NCORES):
        hub_c = hub[c * BL:(c + 1) * BL]  # [BL, W, F]
        hubT = hub_c.transpose(2, 1, 0).reshape(F, W * BL).astype(np.float16)
        hubA = np.vstack([hubT, np.ones((1, W * BL), np.float16)])
        in_maps.append({"hubA": np.ascontiguousarray(hubA), **shared})
    return in_maps, flags


def kernel(**inputs) -> np.ndarray:
    from concourse.bass_utils import run_bass_kernel_spmd

    in_maps, flags = _host_prep(inputs)
    if flags not in _BUILD_CACHE:
        _BUILD_CACHE[flags] = _build_nc(flags)
    nc = _BUILD_CACHE[flags]

    res = run_bass_kernel_spmd(nc, in_maps, core_ids=list(range(NCORES)))
    out = np.empty((B, HOR), np.float32)
    for c in range(NCORES):
        out[c * BL:(c + 1) * BL] = res.results[c]["out"].T
    return out


# revision 29
# speedup vs baseline: 1.0773x; 1.0529x over previous
"""Trainium2 Bass kernel for nn_GCNGRU_Single (SAGEConv x2 on star graph -> 2-layer GRU -> FC).

Algebraic reduction (exact): only the hub node's sequence reaches the output, so
    seq[b,w,:] = (features[b,w,0,:] @ Wr1 + b1) @ Wr2 + b2      (Wl1/Wl2 unused)
    gi0        = seq @ Wih0.T + bih0 = hub_aug @ W_A            (bias via ones-row)

V3 device schedule (per core, batch 16, weights replicated). The per-beat serial
ring is the pacer; it is kept to 4 small ops plus a short matmul segment:

    mm(gi inject + Whh r,z) -> sigmoid -> scan1 -> tanh -> scan2 -> next beat

  * PSUM accumulation groups must be contiguous on the PE queue (measured:
    interleaving other groups between a stopped group and a start=False
    accumulate silently drops the old content), so gi_r/gi_z are injected with
    an I @ GI0 matmul immediately before each Whh matmul, baseline-style.
    The n gate needs NO accumulation: gin lands in its own stride-3 slots.
  * scan1 uses stride-3 groups  d0 = [0, 1, r], d1 = [ghn, 0, gin]:
        y2 = r*ghn + gin = an   (an at offset 2, same offset as r/z in masks)
  * the (h - n) subtract is folded into scan2 via stride-3 groups
        d0 = [0, -1, z]   d1 = [n, h, n]   ->  y2 = z*(h-n) + n = h'
    with tanh double-writing n (stride-0 broadcast read) and the h copy done
    on the Pool engine off the critical path.
  * one sigmoid writes both masks: out = M[:, :, 2::3]  (r row 0, z row 1).
  * L1 lags two beats; its ops fill engine slack behind L0's ring. Its input
    projection pairs (Wih1 first, start=True) execute before h1 arrives.

PSUM beat tile [H, 80] f32: [0:16) a_r | [16:32) a_z | [32:80) n-groups
(3b+32: ghn, 3b+33: 0 (memset once), 3b+34: gin).
"""

import sys

import numpy as np

for _p in ("/opt/trn_rl_repo", "/opt/pypackages"):
    if _p not in sys.path:
        sys.path.append(_p)

B, W, S, F, H, HOR = 128, 64, 64, 64, 128, 12
NCORES = 8
BL = B // NCORES  # 16 batch items per core
FA = F + 1        # hub features + ones row (bias folding)

# Recover the axon terminal if a previous process left a wedged NRT exec unit.
try:
    import ctypes as _ct

    _ct.CDLL("/opt/axon/libaxon_pjrt.so").axon_reset()
except Exception:
    pass

_BUILD_CACHE: dict = {}


def _build_nc(flags):
    import concourse.bacc as bacc
    import concourse.tile as tile
    from concourse import mybir
    from concourse.tile import add_dep_helper

    bhh0n_nz, b1rz_nz, bih1n_nz, bhh1n_nz = flags
    f32 = mybir.dt.float32
    f16 = mybir.dt.float16
    Sig = mybir.ActivationFunctionType.Sigmoid
    Tanh = mybir.ActivationFunctionType.Tanh
    Ident = mybir.ActivationFunctionType.Identity
    MUL = mybir.AluOpType.mult
    ADD = mybir.AluOpType.add

    nc = bacc.Bacc("TRN2", target_bir_lowering=False, debug=False,
                   enable_asserts=False, num_devices=NCORES)

    need_brep = bhh0n_nz or b1rz_nz or bih1n_nz or bhh1n_nz
    hubA_d = nc.dram_tensor("hubA", [FA, W * BL], f16, kind="ExternalInput")
    WA_d = nc.dram_tensor("WA", [FA, 3 * H], f16, kind="ExternalInput")
    WPK = 3 * (3 * H) + H + HOR  # Whh0T | Wih1T | Whh1T | I128 | Wfc
    wpack_d = nc.dram_tensor("wpack", [H, WPK], f16, kind="ExternalInput")
    bfc_d = nc.dram_tensor("bfc", [HOR, 1], f32, kind="ExternalInput")
    if need_brep:
        brep_d = nc.dram_tensor("brep", [H, 5 * BL], f16, kind="ExternalInput")
    out_d = nc.dram_tensor("out", [HOR, BL], f32, kind="ExternalOutput")

    CH1 = 8 * BL                          # hubA columns DMA'd first

    with tile.TileContext(nc) as tc:
        with (
            tc.tile_pool(name="weights", bufs=1) as wpool,
            tc.tile_pool(name="state", bufs=4) as hpool,
            tc.tile_pool(name="an", bufs=3) as anpool,
            tc.tile_pool(name="psBeat", bufs=1, space="PSUM") as psBeat,
            tc.tile_pool(name="psPre", bufs=3, space="PSUM") as psPre,
        ):
            hubA = wpool.tile([FA, W * BL], f16, tag="hubA")
            WA = wpool.tile([FA, 3 * H], f16, tag="WA")
            wpack = wpool.tile([H, WPK], f16, tag="wpack")
            Whh0T = wpack[:, 0:3 * H]
            Wih1T = wpack[:, 3 * H:6 * H]
            Whh1T = wpack[:, 6 * H:9 * H]
            I128 = wpack[:, 9 * H:10 * H]
            Wfc = wpack[:, 10 * H:10 * H + HOR]
            bfc = wpool.tile([HOR, 1], f32, tag="bfc")
            GI0r = wpool.tile([H, W * BL], f16, tag="GI0r")
            GI0z = wpool.tile([H, W * BL], f16, tag="GI0z")
            GI0n = wpool.tile([H, W * BL], f16, tag="GI0n")
            if need_brep:
                brep = wpool.tile([H, 5 * BL], f16, tag="brep")

            # one packed weight DMA per queue; transfers gate beat 0
            # order transfers by when the program first needs them
            nc.sync.dma_start(out=hubA[:, 0:CH1], in_=hubA_d[:, 0:CH1])
            nc.sync.dma_start(out=WA[:], in_=WA_d[:])
            nc.sync.dma_start(out=hubA[:, CH1:], in_=hubA_d[:, CH1:])
            nc.gpsimd.dma_start(out=wpack[:, 9 * H:], in_=wpack_d[:, 9 * H:])
            nc.gpsimd.dma_start(out=wpack[:, 0:3 * H], in_=wpack_d[:, 0:3 * H])
            nc.gpsimd.dma_start(out=wpack[:, 3 * H:9 * H], in_=wpack_d[:, 3 * H:9 * H])
            nc.gpsimd.dma_start(out=bfc[:], in_=bfc_d[:])
            if need_brep:
                nc.gpsimd.dma_start(out=brep[:], in_=brep_d[:])

            # masks: row0 = [0, 1, r], row1 = [0, -1, z] per group of 3
            M0 = [wpool.tile([H, 2, 3 * BL], f16, tag=f"M0{i}", name=f"M0{i}")
                  for i in range(2)]
            M1 = [wpool.tile([H, 2, 3 * BL], f16, tag=f"M1{i}", name=f"M1{i}")
                  for i in range(2)]
            for m in M0 + M1:
                nc.vector.memset(m[:], 0.0)
                nc.vector.memset(m[:, 0, 1:3 * BL:3], 1.0)
                nc.vector.memset(m[:, 1, 1:3 * BL:3], -1.0)

            h_init0 = hpool.tile([H, 3 * BL + 1], f16, tag="hinit0", bufs=1)
            nc.vector.memset(h_init0[:], 0.0)
            h_init1 = hpool.tile([H, 3 * BL + 1], f16, tag="hinit1", bufs=1)
            nc.vector.memset(h_init1[:], 0.0)

            # per-beat PSUM tiles, manually rotated; 33+3b slots stay 0 forever
            P0s = [psBeat.tile([H, 80], f32, tag=f"P0_{i}", name=f"P0_{i}")
                   for i in range(2)]
            P1s = [psBeat.tile([H, 80], f32, tag=f"P1_{i}", name=f"P1_{i}")
                   for i in range(2)]
            for p in P0s + P1s:
                nc.vector.memset(p[:, 33:80:3], 0.0)

            # ---- GI0 precompute: PSUM -> SBUF fp16 (bias via ones-row) ----
            # chunk 0 (steps 0-7) runs before the loop; the remaining 8-step
            # chunks are interleaved into early beats' engine slack below.
            ring_anchor: dict = {}

            def pre_mm(c, g, anchor=None):
                w0c, w1c = 8 * c, 8 * (c + 1)
                ncols = (w1c - w0c) * BL
                pg = psPre.tile([H, 128], f32, tag="pre", name="pg")
                i_mm = nc.tensor.matmul(
                    out=pg[:, 0:ncols],
                    lhsT=WA[:, g * H:(g + 1) * H],
                    rhs=hubA[:, w0c * BL:w1c * BL],
                    start=True, stop=True,
                )
                if anchor is not None:
                    add_dep_helper(i_mm.ins, anchor.ins, sync=True,
                                   reason="precompute chunks stay behind the ring")
                return pg

            def pre_copy(c, g, pg):
                dst = (GI0r, GI0z, GI0n)[g]
                nc.vector.tensor_scalar_add(
                    dst[:, 8 * c * BL:8 * (c + 1) * BL], pg[:, 0:8 * BL], 0.0)

            for g in range(3):
                pg = pre_mm(0, g)
                pre_copy(0, g, pg)

            pre_sched: dict = {}  # beat -> list of emit thunks
            for c in range(1, 8):
                b0 = 1 + (c - 1) * 3
                pgs = {}
                def mk_mm(c=c, b0=b0):
                    def f():
                        anchor = ring_anchor.get(b0 - 1)
                        for g in range(3):
                            pgs[g] = pre_mm(c, g, anchor if g == 0 else None)
                    return f
                pre_sched.setdefault(b0, []).append(mk_mm())
                for g in range(3):
                    def mk_cp(c=c, g=g):
                        return lambda: pre_copy(c, g, pgs[g])
                    pre_sched.setdefault(b0 + g, []).append(mk_cp())

            h0_prev = h_init0
            h0_pprev = h_init0
            h1_prev = h_init1

            for u in range(W + 2):
                do0 = u < W
                do1 = u >= 2
                if do0:
                    P0 = P0s[u % 2]
                    h0r = h0_prev[:, 2:3 * BL:3]
                    c0 = slice(u * BL, (u + 1) * BL)
                    # r, z: contiguous (inject, Whh) accumulation pairs
                    for g, (blk, gi) in enumerate(((slice(0, BL), GI0r),
                                                   (slice(BL, 2 * BL), GI0z))):
                        nc.tensor.matmul(out=P0[:, blk], lhsT=I128,
                                         rhs=gi[:, c0], start=True, stop=False,
                                         skip_group_check=True)
                        nc.tensor.matmul(out=P0[:, blk],
                                         lhsT=Whh0T[:, g * H:(g + 1) * H],
                                         rhs=h0r, start=False, stop=True,
                                         skip_group_check=True)
                    # n: ghn and gin have their own slots, no accumulation
                    if bhh0n_nz:
                        nc.tensor.matmul(out=P0[:, 32:80:3], lhsT=I128,
                                         rhs=brep[:, 0:BL], start=True, stop=False,
                                         skip_group_check=True)
                    nc.tensor.matmul(out=P0[:, 32:80:3], lhsT=Whh0T[:, 2 * H:3 * H],
                                     rhs=h0r, start=not bhh0n_nz, stop=True,
                                     skip_group_check=True)
                    nc.tensor.matmul(out=P0[:, 34:80:3], lhsT=I128,
                                     rhs=GI0n[:, c0], start=True, stop=True,
                                     skip_group_check=True)
                if do1:
                    P1 = P1s[u % 2]
                    h1r = h1_prev[:, 2:3 * BL:3]
                    h0x = h0_pprev[:, 2:3 * BL:3]
                    for g in range(2):
                        blk = slice(g * BL, (g + 1) * BL)
                        # Wih1 part first: its operand is 2 beats old
                        nc.tensor.matmul(out=P1[:, blk],
                                         lhsT=Wih1T[:, g * H:(g + 1) * H],
                                         rhs=h0x, start=True, stop=False,
                                         skip_group_check=True)
                        if b1rz_nz:
                            nc.tensor.matmul(out=P1[:, blk], lhsT=I128,
                                             rhs=brep[:, (1 + g) * BL:(2 + g) * BL],
                                             start=False, stop=False,
                                             skip_group_check=True)
                        nc.tensor.matmul(out=P1[:, blk],
                                         lhsT=Whh1T[:, g * H:(g + 1) * H],
                                         rhs=h1r, start=False, stop=True,
                                         skip_group_check=True)
                    if bhh1n_nz:
                        nc.tensor.matmul(out=P1[:, 32:80:3], lhsT=I128,
                                         rhs=brep[:, 4 * BL:5 * BL], start=True,
                                         stop=False, skip_group_check=True)
                    nc.tensor.matmul(out=P1[:, 32:80:3], lhsT=Whh1T[:, 2 * H:3 * H],
                                     rhs=h1r, start=not bhh1n_nz, stop=True,
                                     skip_group_check=True)
                    nc.tensor.matmul(out=P1[:, 34:80:3], lhsT=Wih1T[:, 2 * H:3 * H],
                                     rhs=h0x, start=True, stop=not bih1n_nz,
                                     skip_group_check=True)
                    if bih1n_nz:
                        nc.tensor.matmul(out=P1[:, 34:80:3], lhsT=I128,
                                         rhs=brep[:, 3 * BL:4 * BL], start=False,
                                         stop=True, skip_group_check=True)

                for thunk in pre_sched.pop(u, ()):
                    thunk()


                def gate_math(P, m, hprev, an_tag, h_tag, sc2_eng=nc.vector):
                    # d1 of scan2 = hprev[1:49]: slot 3b+1 of the view aliases
                    # hprev's h' position (3b+2); tanh overwrites the n slots.
                    un = hprev[:, 1:3 * BL + 1]
                    i_sig = nc.scalar.activation(
                        out=m[:, :, 2:3 * BL:3],
                        in_=P[:, 0:2 * BL].rearrange("p (g b) -> p g b", g=2),
                        func=Sig)
                    an = anpool.tile([H, 3 * BL], f32, tag=an_tag, name=an_tag)
                    nc.vector.tensor_tensor_scan(
                        out=an[:], data0=m[:, 0, :], data1=P[:, 32:80],
                        initial=0.0, op0=MUL, op1=ADD)
                    i_tanh = nc.scalar.activation(
                        out=un.rearrange("p (b t) -> p b t", t=3)[:, :, 0:3:2],
                        in_=an[:, 2:3 * BL:3].unsqueeze(2).broadcast_to([H, BL, 2]),
                        func=Tanh)
                    h_new = hpool.tile([H, 3 * BL + 1], f16, tag=h_tag, name=h_tag)
                    i_sc2 = sc2_eng.tensor_tensor_scan(
                        out=h_new[:, 0:3 * BL], data0=m[:, 1, :], data1=un[:],
                        initial=0.0, op0=MUL, op1=ADD)
                    return h_new, (i_sig, i_tanh), i_sc2

                # NOTE: no explicit Scalar-ordering dep here — the scheduler
                # already orders sig0, tanh1(u-1), tanh0, sig1 well, and the
                # extra semaphore cost ~55ns/beat (measured).
                if do0:
                    h0_new, _, i_sc2 = gate_math(P0, M0[u % 2], h0_prev, "an0", "h0")
                    ring_anchor[u] = i_sc2
                if do1:
                    h1_new, _, _ = gate_math(P1, M1[u % 2], h1_prev, "an1", "h1")

                if do0:
                    h0_pprev = h0_prev
                    h0_prev = h0_new
                else:
                    h0_pprev = h0_prev
                if do1:
                    h1_prev = h1_new

            # ---- final FC: out = Wfc.T @ h1 + bfc ----
            with tc.tile_pool(name="psFC", bufs=1, space="PSUM") as psFC:
                pfc = psFC.tile([HOR, BL], f32, tag="fc")
                nc.tensor.matmul(out=pfc[:], lhsT=Wfc,
                                 rhs=h1_prev[:, 2:3 * BL:3], start=True, stop=True)
                t_out = anpool.tile([HOR, BL], f32, tag="out")
                nc.scalar.activation(out=t_out[:], in_=pfc[:], func=Ident,
                                     bias=bfc[:, 0:1])
                nc.sync.dma_start(out=out_d[:], in_=t_out[:])

    nc.compile()
    return nc


def _host_prep(inputs):
    """Fold weights on host (float64 folds), build per-core input maps."""
    fx = np.asarray(inputs["features"], np.float32)
    Wr1 = np.asarray(inputs["Wr1"], np.float64)
    Wr2 = np.asarray(inputs["Wr2"], np.float64)
    b1 = np.asarray(inputs["b1"], np.float64)
    b2 = np.asarray(inputs["b2"], np.float64)
    Wih0 = np.asarray(inputs["Wih0"], np.float64)
    bih0 = np.asarray(inputs["bih0"], np.float64)
    bhh0 = np.asarray(inputs["bhh0"], np.float64)
    Wih1 = np.asarray(inputs["Wih1"], np.float32)
    Whh0 = np.asarray(inputs["Whh0"], np.float32)
    Whh1 = np.asarray(inputs["Whh1"], np.float32)
    bih1 = np.asarray(inputs["bih1"], np.float64)
    bhh1 = np.asarray(inputs["bhh1"], np.float64)
    Wfc = np.asarray(inputs["Wfc"], np.float32)
    bfc = np.asarray(inputs["bfc"], np.float32)

    W12 = Wr1 @ Wr2                       # [F, H]
    bias12 = b1 @ Wr2 + b2                # [H]
    W_A = W12 @ Wih0.T                    # [F, 3H]
    b_A = (bias12 @ Wih0.T + bih0).copy()  # [3H]
    b_A[0:H] += bhh0[0:H]
    b_A[H:2 * H] += bhh0[H:2 * H]
    WA_aug = np.vstack([W_A, b_A[None, :]]).astype(np.float16)  # [FA, 3H]

    brep = np.zeros((H, 5 * BL), np.float16)
    brep[:, 0 * BL:1 * BL] = bhh0[2 * H:3 * H, None]
    brep[:, 1 * BL:2 * BL] = (bih1[0:H] + bhh1[0:H])[:, None]
    brep[:, 2 * BL:3 * BL] = (bih1[H:2 * H] + bhh1[H:2 * H])[:, None]
    brep[:, 3 * BL:4 * BL] = bih1[2 * H:3 * H, None]
    brep[:, 4 * BL:5 * BL] = bhh1[2 * H:3 * H, None]

    flags = (
        bool(np.any(brep[:, 0:BL] != 0)),
        bool(np.any(brep[:, BL:3 * BL] != 0)),
        bool(np.any(brep[:, 3 * BL:4 * BL] != 0)),
        bool(np.any(brep[:, 4 * BL:5 * BL] != 0)),
    )
    need_brep = any(flags)

    wpack = np.concatenate([
        Whh0.T.astype(np.float16),
        Wih1.T.astype(np.float16),
        Whh1.T.astype(np.float16),
        np.eye(H, dtype=np.float16),
        Wfc.astype(np.float16),
    ], axis=1)
    shared = {
        "WA": np.ascontiguousarray(WA_aug),
        "wpack": np.ascontiguousarray(wpack),
        "bfc": np.ascontiguousarray(bfc.reshape(HOR, 1)),
    }
    if need_brep:
        shared["brep"] = brep

    hub = fx[:, :, 0, :]                  # [B, W, F]
    in_maps = []
    for c in range(NCORES):
        hub_c = hub[c * BL:(c + 1) * BL]  # [BL, W, F]
        hubT = hub_c.transpose(2, 1, 0).reshape(F, W * BL).astype(np.float16)
        hubA = np.vstack([hubT, np.ones((1, W * BL), np.float16)])
        in_maps.append({"hubA": np.ascontiguousarray(hubA), **shared})
    return in_maps, flags


def kernel(**inputs) -> np.ndarray:
    from concourse.bass_utils import run_bass_kernel_spmd

    in_maps, flags = _host_prep(inputs)
    if flags not in _BUILD_CACHE:
        _BUILD_CACHE[flags] = _build_nc(flags)
    nc = _BUILD_CACHE[flags]

    res = run_bass_kernel_spmd(nc, in_maps, core_ids=list(range(NCORES)))
    out = np.empty((B, HOR), np.float32)
    for c in range(NCORES):
        out[c * BL:(c + 1) * BL] = res.results[c]["out"].T
    return out
